# revision 1
# baseline (speedup 1.0000x reference)
"""Multi-head attention Trainium2 kernel (8 NeuronCores, SPMD).

Problem: N=2, Lq=Lk=2048, D=1024, H=16 heads, causal + padding mask,
score scaling = sqrt(#valid keys per sentence).

Sharding: core c -> (n = c // 4, g = c % 4): batch n, head group g of 4
heads (256 feature columns). No cross-core communication; the host
assembles the per-core [2048, 256] outputs into [2, 2048, 1024].

Per-core pipeline (all on one NeuronCore):
  1. xq/xk arrive twice: host-cast bf16 copies are DMA-transposed (xbar,
     2-byte dtype) straight into d-major SBUF tiles xqT/xkT.
  2. Projections (bf16 matmuls, fp32 PSUM): QT/KT [256f, 2048s] bf16 and
     V [2048s, 256f] -> packed bf16 Vtilde [k-chunk][128, 4*65] with a
     ones column per head (softmax denominators for free).
  3. Scores transposed ST[k, q] = KT-slice.T @ QT (row-tiled head pairs,
     two concurrent 64-contraction matmuls). Causal handled structurally
     (skip fully-masked k-chunks and fully-masked q-subblocks) plus one
     [128,128] strictly-lower-triangular additive mask on diagonal
     chunks. The padding mask enters as the scalar-engine activation
     bias (per-partition = per-key). exp on ACT writes bf16 P'T tiles.
     The 1/sqrt(valid) scaling is folded into Wq on the host.
  4. PV: out[q, 0:64]+sum[q] = P'T-chunk.T @ Vtilde (bf16), accumulated
     over k-chunks in PSUM; normalize by the ones-column; DMA out.
"""

import sys

sys.path.insert(0, "/opt/trn_rl_repo")

import numpy as np
import ml_dtypes

import concourse.tile as tile
from concourse import bacc, mybir
from concourse.bass_utils import run_bass_kernel_spmd

F32 = mybir.dt.float32
BF16 = mybir.dt.bfloat16

L = 2048          # sequence length (q and k)
D = 1024          # model dim
FPC = 256         # features per core (4 heads x 64)
HPC = 4           # heads per core
SC = L // 128     # 16 seq chunks of 128
DC = D // 128     # 8 d chunks of 128
NB = L // 512     # 4 q-blocks of 512
NEG = -1.0e9


def build_program(reps=1):
    nc = bacc.Bacc("TRN2", target_bir_lowering=False, debug=False, num_devices=8)

    xq_d = nc.dram_tensor("xq_bf", [L, D], BF16, kind="ExternalInput").ap()
    xk_d = nc.dram_tensor("xk_bf", [L, D], BF16, kind="ExternalInput").ap()
    wq_d = nc.dram_tensor("wq_t", [D, FPC], BF16, kind="ExternalInput").ap()
    wk_d = nc.dram_tensor("wk_t", [D, FPC], BF16, kind="ExternalInput").ap()
    wv_d = nc.dram_tensor("wv_t", [D, FPC], BF16, kind="ExternalInput").ap()
    pb_d = nc.dram_tensor("pad_bias", [128, SC], F32, kind="ExternalInput").ap()
    out_d = nc.dram_tensor("out", [L, FPC], F32, kind="ExternalOutput").ap()

    with tile.TileContext(nc) as tc:
        with (
            tc.tile_pool(name="consts", bufs=1) as consts,
            tc.tile_pool(name="wpool", bufs=1) as wpool,
            tc.tile_pool(name="xt", bufs=1) as xt_pool,
            tc.tile_pool(name="qkv", bufs=1) as qkv,
            tc.tile_pool(name="pt", bufs=34) as pt_pool,
            tc.tile_pool(name="ostage", bufs=2) as out_pool,
            tc.tile_pool(name="small", bufs=4) as small_pool,
            tc.tile_pool(name="proj", bufs=2, space="PSUM") as proj_pool,
            tc.tile_pool(name="stps", bufs=2, space="PSUM") as st_pool,
            tc.tile_pool(name="pvps", bufs=2, space="PSUM") as pv_pool,
        ):
          for _rep in range(reps):
            # diag_mask[i, j] = NEG where j < i else 0  (strictly-lower tri)
            diag_mask = consts.tile([128, 128], F32)
            nc.gpsimd.memset(diag_mask, 0.0)
            nc.gpsimd.affine_select(
                out=diag_mask,
                in_=diag_mask,
                compare_op=mybir.AluOpType.is_ge,
                fill=NEG,
                base=0,
                pattern=[[1, 128]],
                channel_multiplier=-1,
            )
            pad_bias = consts.tile([128, SC], F32)
            nc.sync.dma_start(out=pad_bias, in_=pb_d)

            # weights: [128 (d within chunk), dc, f]
            wq = wpool.tile([128, DC, FPC], BF16)
            wk = wpool.tile([128, DC, FPC], BF16)
            wv = wpool.tile([128, DC, FPC], BF16)
            for w_sb, w_dr in ((wq, wq_d), (wk, wk_d), (wv, wv_d)):
                nc.sync.dma_start(
                    out=w_sb, in_=w_dr.rearrange("(dc p) f -> p dc f", p=128)
                )

            # ACT warmup: trigger the exp table load at t~0 so the first
            # real exp doesn't pay the ~2.7us LoadActFuncSet latency.
            warm = small_pool.tile([128, 1], F32, tag="warm")
            warm2 = small_pool.tile([128, 1], F32, tag="warm")
            nc.vector.memset(warm, 0.0)
            nc.scalar.activation(warm2, warm, mybir.ActivationFunctionType.Exp)

            # x transposed, d-major: [128 (d in chunk), dc, seq]
            # Emitted in consumption order: projections for slab sb need all
            # d-chunks of BOTH xq and xk for that slab.
            xqt = xt_pool.tile([128, DC, L], BF16)
            xkt = xt_pool.tile([128, DC, L], BF16)
            for sb in range(4):
                for x_d, x_t in ((xq_d, xqt), (xk_d, xkt)):
                    for dc in range(DC):
                        nc.sync.dma_start(
                            out=x_t[:, dc, 512 * sb : 512 * (sb + 1)],
                            in_=x_d[
                                512 * sb : 512 * (sb + 1),
                                128 * dc : 128 * (dc + 1),
                            ],
                            transpose=True,
                        )

            # projection outputs
            qt = qkv.tile([128, 2, L], BF16)   # [f within chunk, fc, q]
            kt = qkv.tile([128, 2, L], BF16)   # [f within chunk, fc, k]
            vt = qkv.tile([128, SC, HPC * 65], BF16)  # [k in chunk, kc, h*65+f]
            nc.vector.memset(vt, 1.0)  # ones columns (col 64 of each head)

            # ---- helpers -------------------------------------------------
            def proj_slab(sb):
                for fc in range(2):
                    pq = proj_pool.tile([128, 512], F32, tag="proj")
                    for dc in range(DC):
                        nc.tensor.matmul(
                            pq,
                            lhsT=wq[:, dc, 128 * fc : 128 * (fc + 1)],
                            rhs=xqt[:, dc, 512 * sb : 512 * (sb + 1)],
                            start=(dc == 0),
                            stop=(dc == DC - 1),
                        )
                    nc.vector.tensor_copy(qt[:, fc, 512 * sb : 512 * (sb + 1)], pq)
                for fc in range(2):
                    pk = proj_pool.tile([128, 512], F32, tag="proj")
                    for dc in range(DC):
                        nc.tensor.matmul(
                            pk,
                            lhsT=wk[:, dc, 128 * fc : 128 * (fc + 1)],
                            rhs=xkt[:, dc, 512 * sb : 512 * (sb + 1)],
                            start=(dc == 0),
                            stop=(dc == DC - 1),
                        )
                    nc.vector.tensor_copy(kt[:, fc, 512 * sb : 512 * (sb + 1)], pk)
                # V = xkT.T @ wv   -> [k-seq, f]
                for i in range(4):
                    kc = 4 * sb + i
                    pv = proj_pool.tile([128, 512], F32, tag="proj")
                    for dc in range(DC):
                        nc.tensor.matmul(
                            pv[:, 0:FPC],
                            lhsT=xkt[:, dc, 128 * kc : 128 * (kc + 1)],
                            rhs=wv[:, dc, :],
                            start=(dc == 0),
                            stop=(dc == DC - 1),
                        )
                    # scatter heads into vt (col 64 of each head stays 1.0)
                    nc.vector.tensor_copy(
                        vt[:, kc, :].rearrange("p (h f) -> p h f", h=HPC)[
                            :, :, 0:64
                        ],
                        pv[:, 0:FPC].rearrange("p (h f) -> p h f", h=HPC),
                    )

            def st_exp(b, c, pts):
                qs = max(0, c - 4 * b) * 128  # skip fully-masked q cols
                width = 512 - qs
                for p in range(2):  # head pair = feature chunk
                    st = st_pool.tile([128, 2, 512], F32, tag="st")
                    for hh in range(2):
                        lo, hi = 64 * hh, 64 * (hh + 1)
                        nc.tensor.matmul(
                            st[:, hh, :],
                            lhsT=kt[lo:hi, p, 128 * c : 128 * (c + 1)],
                            rhs=qt[lo:hi, p, 512 * b : 512 * (b + 1)],
                            start=True,
                            stop=True,
                        )
                    if c >= 4 * b:
                        j = c - 4 * b
                        for hh in range(2):
                            sl = st[:, hh, 128 * j : 128 * (j + 1)]
                            nc.vector.tensor_add(sl, sl, diag_mask)
                    pt = pt_pool.tile([128, 2, width], BF16, tag="pt")
                    nc.scalar.activation(
                        pt,
                        st[:, :, qs:],
                        mybir.ActivationFunctionType.Exp,
                        bias=pad_bias[:, c : c + 1],
                        scale=1.0,
                    )
                    pts[(c, p)] = (pt, qs)

            def pv_qchunk(b, j, pts):
                qc = 4 * b + j
                ostage = out_pool.tile([128, FPC], F32, tag="os")
                for h in range(HPC):
                    p, hh = h // 2, h % 2
                    po = pv_pool.tile([128, 65], F32, tag="po")
                    for c in range(qc + 1):
                        ptile, qs = pts[(c, p)]
                        lo = 128 * j - qs
                        nc.tensor.matmul(
                            po,
                            lhsT=ptile[:, hh, lo : lo + 128],
                            rhs=vt[:, c, 65 * h : 65 * (h + 1)],
                            start=(c == 0),
                            stop=(c == qc),
                        )
                    rec = small_pool.tile([128, 1], F32, tag="rec")
                    nc.vector.reciprocal(rec, po[:, 64:65])
                    nc.vector.tensor_scalar_mul(
                        ostage[:, 64 * h : 64 * (h + 1)], po[:, 0:64], rec
                    )
                nc.sync.dma_start(
                    out=out_d[128 * qc : 128 * (qc + 1), :], in_=ostage
                )

            # ---- interleaved schedule: projections feed attention blocks;
            # within a block, PV(j) is emitted right after its last needed
            # exp so the PE never waits a whole block on ACT.
            for b in range(NB):
                proj_slab(b)
                pts = {}
                for c in range(4 * b + 1):
                    st_exp(b, c, pts)
                pv_qchunk(b, 0, pts)
                for j in range(1, 4):
                    st_exp(b, 4 * b + j, pts)
                    pv_qchunk(b, j, pts)

    nc.compile()
    return nc


_NC_CACHE = None


def get_program():
    global _NC_CACHE
    if _NC_CACHE is None:
        _NC_CACHE = build_program()
    return _NC_CACHE


def make_in_maps(query, key, Wq, Wk, Wv, padding_mask):
    query = np.asarray(query, dtype=np.float32)
    key = np.asarray(key, dtype=np.float32)
    Wq = np.asarray(Wq, dtype=np.float32)
    Wk = np.asarray(Wk, dtype=np.float32)
    Wv = np.asarray(Wv, dtype=np.float32)
    padding_mask = np.asarray(padding_mask)
    bf = ml_dtypes.bfloat16

    in_maps = []
    for core in range(8):
        n, g = core // 4, core % 4
        valid = float((~padding_mask[n]).sum())
        inv_scale = 1.0 / np.sqrt(valid)
        sl = slice(g * FPC, (g + 1) * FPC)
        pad_bias = np.where(padding_mask[n], NEG, 0.0).astype(np.float32)
        in_maps.append(
            {
                "xq_bf": np.ascontiguousarray(query[n]).astype(bf),
                "xk_bf": np.ascontiguousarray(key[n]).astype(bf),
                "wq_t": np.ascontiguousarray((Wq[sl] * inv_scale).T).astype(bf),
                "wk_t": np.ascontiguousarray(Wk[sl].T).astype(bf),
                "wv_t": np.ascontiguousarray(Wv[sl].T).astype(bf),
                "pad_bias": np.ascontiguousarray(pad_bias.reshape(SC, 128).T),
            }
        )
    return in_maps


def kernel(query, key, Wq, Wk, Wv, mask, padding_mask, n_heads):
    nc = get_program()
    in_maps = make_in_maps(query, key, Wq, Wk, Wv, padding_mask)
    res = run_bass_kernel_spmd(nc, in_maps, core_ids=list(range(8)))
    out = np.empty((2, L, D), dtype=np.float32)
    for core in range(8):
        n, g = core // 4, core % 4
        out[n, :, g * FPC : (g + 1) * FPC] = res.results[core]["out"]
    return out



# revision 48
# speedup vs baseline: 14689.8070x; 14689.8070x over previous
"""Multi-head attention Trainium2 kernel (8 NeuronCores, SPMD).

Problem: N=2, Lq=Lk=2048, D=1024, H=16 heads, causal + padding mask,
score scaling = sqrt(#valid keys per sentence).

Sharding: core c -> (n = c // 4, g = c % 4): batch n, head group g of 4
heads (256 feature columns). No cross-core communication; the host
assembles the per-core [2048, 256] outputs into [2, 2048, 1024].

Per-core pipeline (all on one NeuronCore):
  1. xq/xk arrive host-pre-transposed ([D, L] bf16): plain fast DMAs
     land d-major SBUF tiles directly (no xbar transposes). Front-end
     DMA issue is split across the two HWDGE rings (sync + scalar).
  2. A stream of dummy matmuls warms the PE HAM clock governor (cold
     1.2 GHz, warm 2.4 GHz; it re-throttles after any ~3.4us activity
     window with idle in it) while the first tiles stream in.
  3. Projections (bf16 matmuls, fp32 PSUM): QT/KT [256f, 2048s] bf16,
     V packed into bf16 Vtilde [kc][128, 4*65] with a ones column per
     head (softmax denominators for free).
  4. Scores ST[k, q] = KT-slice.T @ QT, exp on ACT (padding mask as
     the per-partition ACT bias; 1/sqrt(valid) folded into Wq on the
     host; causal diagonal = triangular zero of the exp output via
     gpsimd affine_select, off the st->exp critical path). The
     schedule is a cross-block software pipeline: each block's
     non-diagonal score chunks interleave into the K/V projection
     stream and the PREVIOUS block's PV phase, so the scalar engine
     enters every block with its exp backlog drained and the PE never
     idles long enough to re-throttle. Explicit dummy-matmul padding
     covers the two structurally ACT-bound pockets (C_0, A_3).
  5. PV: out[q, 0:64]+denom[q] = P'T-chunk.T @ Vtilde accumulated over
     k-chunks in PSUM (one bank per open chain - PSUM allows only one
     open accumulation group per 2KB bank; per chain the diagonal
     chunk is accumulated LAST so its exp hides behind bulk work);
     normalize by the ones column; store per head-pair.
"""

import sys

sys.path.insert(0, "/opt/trn_rl_repo")

import numpy as np
import ml_dtypes

import concourse.tile as tile
from concourse import bacc, mybir
from concourse.bass_utils import run_bass_kernel_spmd

F32 = mybir.dt.float32
BF16 = mybir.dt.bfloat16

L = 2048          # sequence length (q and k)
D = 1024          # model dim
FPC = 256         # features per core (4 heads x 64)
HPC = 4           # heads per core
SC = L // 128     # 16 seq chunks of 128
DC = D // 128     # 8 d chunks of 128
NB = L // 512     # 4 q-blocks of 512
NEG = -1.0e9
N_WARM = 30       # dummy matmuls to hold PE busy until real data lands


def build_program(reps=1):
    nc = bacc.Bacc("TRN2", target_bir_lowering=False, debug=False, num_devices=8)

    # host pre-transposed: d-major [D, L]
    xq_d = nc.dram_tensor("xq_bf", [D, L], BF16, kind="ExternalInput").ap()
    xk_d = nc.dram_tensor("xk_bf", [D, L], BF16, kind="ExternalInput").ap()
    wq_d = nc.dram_tensor("wq_t", [D, FPC], BF16, kind="ExternalInput").ap()
    wk_d = nc.dram_tensor("wk_t", [D, FPC], BF16, kind="ExternalInput").ap()
    wv_d = nc.dram_tensor("wv_t", [D, FPC], BF16, kind="ExternalInput").ap()
    pb_d = nc.dram_tensor("pad_bias", [128, SC], F32, kind="ExternalInput").ap()
    out_d = nc.dram_tensor("out", [L, FPC], F32, kind="ExternalOutput").ap()

    with tile.TileContext(nc) as tc:
        with (
            tc.tile_pool(name="consts", bufs=1) as consts,
            tc.tile_pool(name="wpool", bufs=1) as wpool,
            tc.tile_pool(name="xt", bufs=1) as xt_pool,
            tc.tile_pool(name="qkv", bufs=1) as qkv,
            tc.tile_pool(name="pt", bufs=34) as pt_pool,
            tc.tile_pool(name="ostage", bufs=3) as out_pool,
            tc.tile_pool(name="small", bufs=4) as small_pool,
            tc.tile_pool(name="pstage", bufs=3) as pstage_pool,
            tc.tile_pool(name="ring", bufs=2, space="PSUM") as ring,
            tc.tile_pool(name="stps", bufs=3, space="PSUM") as st_pool,
        ):
          for _rep in range(reps):
            # ---- front end ------------------------------------------------
            # ACT warmup: trigger the exp table load at t~0 so the first
            # real exp doesn't pay the table-load latency. Issued on
            # scalar BEFORE its front-end DMA triggers.
            warm = small_pool.tile([128, 1], F32, tag="warm")
            warm2 = small_pool.tile([128, 1], F32, tag="warm")
            nc.vector.memset(warm, 0.0)
            nc.scalar.activation(warm2, warm, mybir.ActivationFunctionType.Exp)

            # weights: [128 (d within chunk), dc, f]
            wq = wpool.tile([128, DC, FPC], BF16)
            wk = wpool.tile([128, DC, FPC], BF16)
            wv = wpool.tile([128, DC, FPC], BF16)

            # x transposed, d-major: [128 (d in chunk), dc, seq]
            xqt = xt_pool.tile([128, DC, L], BF16)
            xkt = xt_pool.tile([128, DC, L], BF16)

            def load_x(eng, x_d, x_t, sb):
                for dc in range(DC):
                    eng.dma_start(
                        out=x_t[:, dc, 512 * sb : 512 * (sb + 1)],
                        in_=x_d[128 * dc : 128 * (dc + 1), 512 * sb : 512 * (sb + 1)],
                    )

            # sync HWDGE ring: wq then xq slab 0 (Q proj runs first)
            nc.sync.dma_start(out=wq, in_=wq_d.rearrange("(dc p) f -> p dc f", p=128))
            load_x(nc.sync, xq_d, xqt, 0)
            # scalar HWDGE ring: pad bias FIRST (it seeds the PE warmup
            # and HWDGE transfers are FIFO per ring), then wk, wv, xk0
            pad_bias = consts.tile([128, SC], F32)
            nc.scalar.dma_start(out=pad_bias, in_=pb_d)
            nc.scalar.dma_start(out=wk, in_=wk_d.rearrange("(dc p) f -> p dc f", p=128))
            nc.scalar.dma_start(out=wv, in_=wv_d.rearrange("(dc p) f -> p dc f", p=128))
            load_x(nc.scalar, xk_d, xkt, 0)

            # PE warmup: dummy matmuls so HAM un-throttles (~3.4us of
            # SUSTAINED busy) before the first projection and stays warm
            # until real data lands. Dead PSUM writes, never read.
            # Stage 1: tiny matmuls on pad_bias (earliest-arriving tensor,
            # ~4us) - activity only, too low duty-cycle to warm HAM.
            for i in range(8):
                wu_ps = ring.tile([16, 16], F32, tag="bank", name="wu_ps")
                nc.tensor.matmul(
                    wu_ps, lhsT=pad_bias[:, 0:16], rhs=pad_bias,
                    start=True, stop=True,
                )
            # Stage 2: N=256 matmuls on wq (arrives ~6.5us) - dense
            # enough to trip the HAM busy window before Q proj starts.
            for i in range(N_WARM):
                wu_ps = ring.tile([128, 256], F32, tag="bank", name="wu_ps")
                nc.tensor.matmul(
                    wu_ps, lhsT=wq[:, 0, 0:128], rhs=wq[:, 0, :],
                    start=True, stop=True,
                )

            # bf16 padder source for mid-kernel HAM-hold dummies
            wu_src = consts.tile([128, 512], BF16)
            nc.gpsimd.memset(wu_src, 0.0)

            def pad_pe(n):
                # dead matmuls: PE occupancy filler for ACT-bound pockets
                # (a HAM idle-window re-throttle costs ~2x what these do)
                for _ in range(n):
                    wu = ring.tile([128, 512], F32, tag="bank", name="wu")
                    nc.tensor.matmul(
                        wu, lhsT=wu_src[:, 0:128], rhs=wu_src,
                        start=True, stop=True,
                    )

            # projection outputs
            qt = qkv.tile([128, 2, L], BF16)   # [f within chunk, fc, q]
            kt = qkv.tile([128, 2, L], BF16)   # [f within chunk, fc, k]
            vt = qkv.tile([128, SC, HPC * 65], BF16)  # [k in chunk, kc, h*65+f]
            nc.gpsimd.memset(vt, 1.0)  # ones columns (col 64 of each head)

            # ---- helpers -------------------------------------------------
            def qproj_fc(sb, fc):
                pq = ring.tile([128, 512], F32, tag="bank", name="pq")
                for dc in range(DC):
                    nc.tensor.matmul(
                        pq,
                        lhsT=wq[:, dc, 128 * fc : 128 * (fc + 1)],
                        rhs=xqt[:, dc, 512 * sb : 512 * (sb + 1)],
                        start=(dc == 0),
                        stop=(dc == DC - 1),
                    )
                nc.vector.tensor_copy(qt[:, fc, 512 * sb : 512 * (sb + 1)], pq)

            def kproj_fc(sb, fc):
                pk = ring.tile([128, 512], F32, tag="bank", name="pk")
                for dc in range(DC):
                    nc.tensor.matmul(
                        pk,
                        lhsT=wk[:, dc, 128 * fc : 128 * (fc + 1)],
                        rhs=xkt[:, dc, 512 * sb : 512 * (sb + 1)],
                        start=(dc == 0),
                        stop=(dc == DC - 1),
                    )
                nc.vector.tensor_copy(kt[:, fc, 512 * sb : 512 * (sb + 1)], pk)

            def vproj_kc(kc):
                # V = xkT.T @ wv   -> [k-seq, f]
                pv = ring.tile([128, 512], F32, tag="bank", name="pv")
                for dc in range(DC):
                    nc.tensor.matmul(
                        pv[:, 0:FPC],
                        lhsT=xkt[:, dc, 128 * kc : 128 * (kc + 1)],
                        rhs=wv[:, dc, :],
                        start=(dc == 0),
                        stop=(dc == DC - 1),
                    )
                # scatter heads into vt (col 64 of each head stays 1.0)
                nc.vector.tensor_copy(
                    vt[:, kc, :].rearrange("p (h f) -> p h f", h=HPC)[:, :, 0:64],
                    pv[:, 0:FPC].rearrange("p (h f) -> p h f", h=HPC),
                )

            def st_exp(b, c, pts):
                qs = max(0, c - 4 * b) * 128  # skip fully-masked q cols
                width = 512 - qs
                for p in range(2):  # head pair = feature chunk
                    st = st_pool.tile([128, 2, 512], F32, tag="st")
                    for hh in range(2):
                        lo, hi = 64 * hh, 64 * (hh + 1)
                        nc.tensor.matmul(
                            st[:, hh, :],
                            lhsT=kt[lo:hi, p, 128 * c : 128 * (c + 1)],
                            rhs=qt[lo:hi, p, 512 * b : 512 * (b + 1)],
                            start=True,
                            stop=True,
                        )
                    pt = pt_pool.tile([128, 2, width], BF16, tag="pt")
                    nc.scalar.activation(
                        pt,
                        st[:, :, qs:],
                        mybir.ActivationFunctionType.Exp,
                        bias=pad_bias[:, c : c + 1],
                        scale=1.0,
                    )
                    if c >= 4 * b:
                        # causal: zero exp output where q < k inside the
                        # diagonal 128x128 tile (gpsimd, SBUF-only)
                        for hh in range(2):
                            sl = pt[:, hh, 0:128]
                            nc.gpsimd.affine_select(
                                out=sl,
                                in_=sl,
                                compare_op=mybir.AluOpType.is_ge,
                                fill=0.0,
                                base=0,
                                pattern=[[1, 128]],
                                channel_multiplier=-1,
                            )
                    pts[(c, p)] = (pt, qs)

            def pv_qchunk(b, j, pts):
                # Per head pair: bulk accumulation (c < qc) for both heads
                # first, the diagonal-chunk matmuls last, so the diagonal
                # exp latency hides behind the other head's bulk work.
                # PSUM is drained by a cheap copy into pstage (frees the
                # ring slot fast); normalization runs off SBUF afterwards.
                qc = 4 * b + j
                pstage = pstage_pool.tile([128, HPC, 65], F32, tag="ps")
                ostage = out_pool.tile([128, FPC], F32, tag="os")
                for hp in range(2):
                    pos = []
                    for h in (2 * hp, 2 * hp + 1):
                        p, hh = h // 2, h % 2
                        po = ring.tile([128, 65], F32, tag="bank", name="po")
                        for c in range(qc):
                            ptile, qs = pts[(c, p)]
                            lo = 128 * j - qs
                            nc.tensor.matmul(
                                po,
                                lhsT=ptile[:, hh, lo : lo + 128],
                                rhs=vt[:, c, 65 * h : 65 * (h + 1)],
                                start=(c == 0),
                                stop=False,
                            )
                        pos.append((h, po))
                    for h, po in pos:
                        p, hh = h // 2, h % 2
                        ptile, qs = pts[(qc, p)]
                        nc.tensor.matmul(
                            po,
                            lhsT=ptile[:, hh, 0:128],
                            rhs=vt[:, qc, 65 * h : 65 * (h + 1)],
                            start=(qc == 0),
                            stop=True,
                        )
                        nc.vector.tensor_copy(pstage[:, h, :], po)
                    # normalize + store this head pair now (halves the
                    # final-chunk tail: the DMA overlaps hp=1's chains)
                    h0 = 2 * hp
                    rec = small_pool.tile([128, 2], F32, tag="rec")
                    nc.vector.reciprocal(rec, pstage[:, h0 : h0 + 2, 64])
                    for hh2 in range(2):
                        h = h0 + hh2
                        nc.vector.tensor_scalar_mul(
                            ostage[:, 64 * h : 64 * (h + 1)],
                            pstage[:, h, 0:64],
                            rec[:, hh2 : hh2 + 1],
                        )
                    nc.sync.dma_start(
                        out=out_d[
                            128 * qc : 128 * (qc + 1), 128 * hp : 128 * (hp + 1)
                        ],
                        in_=ostage[:, 128 * hp : 128 * (hp + 1)],
                    )

            # ---- schedule ------------------------------------------------
            # Cross-block software pipeline. Per block b:
            #   A phase: K/V projections, interleaved with this block's
            #     early (fully-unmasked) score chunks so ACT chews its
            #     backlog while the PE projects.
            #   C phase: diagonal score chunk + PV accumulation chain per
            #     q-chunk; between chains, "filler" PE work from block
            #     b+1 (its Q projection and first early score chunks) so
            #     the PE never idles out a HAM window waiting on exp.
            pts = [dict() for _ in range(NB)]
            # early score chunks (c < 4b) remaining per block
            early_rem = [list(range(4 * b)) for b in range(NB)]
            kp_hoisted = set()

            def mk_st(bb, cc):
                return lambda: st_exp(bb, cc, pts[bb])

            qproj_fc(0, 0)
            qproj_fc(0, 1)
            for b in range(NB):
                # A phase: remaining V (and first-block K) projections,
                # interleaved with this block's remaining early score
                # chunks, then next block's Q projection at the end.
                if b + 1 < NB:
                    load_x(nc.sync, xq_d, xqt, b + 1)
                projs = (
                    []
                    if b in kp_hoisted
                    else [lambda: kproj_fc(b, 0), lambda: kproj_fc(b, 1)]
                ) + [
                    (lambda kc: lambda: vproj_kc(kc))(4 * b + i) for i in range(4)
                ]
                n_proj = len(projs)
                for i, pr in enumerate(projs):
                    pr()
                    want = (len(early_rem[b]) + (n_proj - i) - 1) // (n_proj - i)
                    for _ in range(min(want, 2)):
                        if early_rem[b]:
                            st_exp(b, early_rem[b].pop(0), pts[b])
                            if b == NB - 1:
                                pad_pe(2)  # A_3 is ACT-bound: hold HAM warm
                while early_rem[b]:
                    st_exp(b, early_rem[b].pop(0), pts[b])
                    if b == NB - 1:
                        pad_pe(2)

                # C phase: diagonal chunk + PV per q-chunk; fillers from
                # block b+1 keep the PE busy through the diag exp waits.
                if b + 1 < NB:
                    load_x(nc.sync, xk_d, xkt, b + 1)
                fillers = [[] for _ in range(4)]
                if b + 1 < NB:
                    nxt = early_rem[b + 1]
                    if b >= 2:
                        # block 3's exp backlog exceeds A_3's PE work:
                        # pull six early chunks forward (Q proj pair
                        # first so the chunks have qt available)
                        fillers[0] += [
                            lambda: qproj_fc(b + 1, 0),
                            lambda: qproj_fc(b + 1, 1),
                        ]
                        for slot in (1, 2, 3, 1, 2, 3):
                            if len(nxt) > 4:
                                fillers[slot].append(mk_st(b + 1, nxt.pop(0)))
                    else:
                        fillers[0].append(lambda: qproj_fc(b + 1, 0))
                        fillers[1].append(lambda: qproj_fc(b + 1, 1))
                        if b == 0:
                            # C_0 is ACT-bound (tiny PV chains): hoist
                            # block 1's K projection in as PE filler.
                            fillers[2].append(lambda: kproj_fc(1, 0))
                            fillers[3].append(lambda: kproj_fc(1, 1))
                            kp_hoisted.add(1)
                        for slot in (2, 3, 2, 3):
                            if nxt:
                                fillers[slot].append(mk_st(b + 1, nxt.pop(0)))
                for j in range(4):
                    st_exp(b, 4 * b + j, pts[b])
                    for f in fillers[j]:
                        f()
                    if b == 0:
                        pad_pe(2)  # C_0 is ACT-bound: hold HAM warm
                    pv_qchunk(b, j, pts[b])

    nc.compile()
    return nc


_NC_CACHE = None


def get_program():
    global _NC_CACHE
    if _NC_CACHE is None:
        _NC_CACHE = build_program()
    return _NC_CACHE


def make_in_maps(query, key, Wq, Wk, Wv, padding_mask):
    query = np.asarray(query, dtype=np.float32)
    key = np.asarray(key, dtype=np.float32)
    Wq = np.asarray(Wq, dtype=np.float32)
    Wk = np.asarray(Wk, dtype=np.float32)
    Wv = np.asarray(Wv, dtype=np.float32)
    padding_mask = np.asarray(padding_mask)
    bf = ml_dtypes.bfloat16

    # per-batch host pre-transpose (shared across the 4 head-group cores)
    xqT = [np.ascontiguousarray(query[n].T).astype(bf) for n in range(2)]
    xkT = [np.ascontiguousarray(key[n].T).astype(bf) for n in range(2)]

    in_maps = []
    for core in range(8):
        n, g = core // 4, core % 4
        valid = float((~padding_mask[n]).sum())
        inv_scale = 1.0 / np.sqrt(valid)
        sl = slice(g * FPC, (g + 1) * FPC)
        pad_bias = np.where(padding_mask[n], NEG, 0.0).astype(np.float32)
        in_maps.append(
            {
                "xq_bf": xqT[n],
                "xk_bf": xkT[n],
                "wq_t": np.ascontiguousarray((Wq[sl] * inv_scale).T).astype(bf),
                "wk_t": np.ascontiguousarray(Wk[sl].T).astype(bf),
                "wv_t": np.ascontiguousarray(Wv[sl].T).astype(bf),
                "pad_bias": np.ascontiguousarray(pad_bias.reshape(SC, 128).T),
            }
        )
    return in_maps


def kernel(query, key, Wq, Wk, Wv, mask, padding_mask, n_heads):
    nc = get_program()
    in_maps = make_in_maps(query, key, Wq, Wk, Wv, padding_mask)
    res = run_bass_kernel_spmd(nc, in_maps, core_ids=list(range(8)))
    out = np.empty((2, L, D), dtype=np.float32)
    for core in range(8):
        n, g = core // 4, core % 4
        out[n, :, g * FPC : (g + 1) * FPC] = res.results[core]["out"]
    return out
